# revision 25
# baseline (speedup 1.0000x reference)
"""AtomPosGNN Trainium2 kernel: 4-layer GraphConv (norm='both') over a dense
0/1 adjacency, SPMD across 8 NeuronCores.

Sharding: nodes split 1024/core. Core m holds the full-height column block
A[:, m*1024:(m+1)*1024] (== row block transposed; A symmetric) as exact 0/1
fp8e4m3, resident in SBUF, rows reordered into rotated rank order
(m+1)%8, ..., (m+7)%8, m (own block last). Layers 1-3 aggregate with fp8
DoubleRow matmuls (two 128-row source chunks per instruction, ~1.5-2x PE
throughput): stationary operand = feature chunk pair [128, 2, 128ch],
moving operand = adjacency chunk pair [128, 2, 512dst].

Host precomputes the degree norms r = rsqrt(max(deg,1)) (graph setup, same
as dgl) and the pre-scaled input features z0 = 16*r*[emb|pos], replicated
to every core in its rotated row order — layer 0 needs no collective and
starts immediately; it runs in bf16 (plain matmuls) since the startup
window is gated by the first collective's entry barrier anyway, and bf16
improves precision for free. Later layers' features are fp8, scaled by 16
to stay out of the subnormal range; the 1/16 is folded into the dst-scale
vector rbc. A tiny dummy AllGather fires first so the one-time CC entry
barrier overlaps the adjacency load and layer 0.

Layers 1-3 gather features via fp8 AllGathers split into two column halves;
each layer's own-block (local) aggregation for all channels is emitted ahead
of the gathered phases so it fills the AllGather wait, and the second half's
collective flies under the first half's aggregation phase. Gathered rank
blocks are fetched with per-core dynamic (register) DMA offsets so every
core skips its own block without branching.

softplus = ln(exp(y)+1) on the ACT engine; the activation-table selection is
patched so both EXP and LN resolve to the combined natural_log_exp table —
one table load for the whole kernel instead of a ~1.3us reload per function
switch.
"""

import numpy as np
import ml_dtypes

N = 8192
NCORES = 8
GRP = ((0, 2), (2, 8))   # row-group split: AllGather 0 is small and early
L = N // NCORES          # 1024 local nodes per core
EMB = 125
POS = 3
IN = 128                 # EMB + POS
H = 512
HH = H // 2              # column half for the split AllGather
RJ = L // 128            # 8 row chunks per rank block
NJ = L // 512            # 2 free-dim chunks of 512 in aggregation
NOTH = NCORES - 1        # 7 gathered (off-rank) blocks
NCH = N // 128           # 64 source chunks total
ZS = 16.0                # fp8 feature scale (keeps z out of subnormals)

BF16 = ml_dtypes.bfloat16
F8 = ml_dtypes.float8_e4m3fn

_STATE = {}


def _patch_act_tables():
    """Make the act-table pass pick the combined exp+ln table for both EXP
    and LN (greedy first-match otherwise alternates two tables, reloading
    ~1.3us per switch). Empties the tables before the combined one so ids
    stay positional."""
    import concourse.bacc as bm
    import concourse.hw_specs as hw
    if getattr(bm, "_gnn_act_patch", False):
        return
    orig = hw.get_activation_tables

    def patched(arch):
        t = orig(arch)
        names = list(t.keys())
        if "natural_log_exp_and_others" not in names:
            return t
        i = names.index("natural_log_exp_and_others")
        return {n: (set() if k < i else t[n]) for k, n in enumerate(names)}

    bm.get_activation_tables = patched
    bm._gnn_act_patch = True


def _build(use_bias):
    import concourse.bass as bass
    import concourse.mybir as mybir
    import concourse.tile as tile
    from concourse import bacc
    from concourse.bass import ds

    _patch_act_tables()

    f32 = mybir.dt.float32
    bf16 = mybir.dt.bfloat16
    fp8 = mybir.dt.float8e4
    u32 = mybir.dt.uint32
    EXP = mybir.ActivationFunctionType.Exp
    LN = mybir.ActivationFunctionType.Ln
    DR = mybir.MatmulPerfMode.DoubleRow

    nc = bacc.Bacc("TRN2", target_bir_lowering=False, debug=False,
                   num_devices=NCORES)

    a_dram = nc.declare_dram_parameter("a", [N, L], fp8, isOutput=False)
    z0_dram = nc.declare_dram_parameter("z0", [N, IN], bf16, isOutput=False)
    rbc_dram = nc.declare_dram_parameter("rbc", [128, L], f32, isOutput=False)
    rp_dram = nc.declare_dram_parameter("rp", [128, RJ], f32, isOutput=False)
    w0_dram = nc.declare_dram_parameter("w0", [IN, H], bf16, isOutput=False)
    wx_dram = [nc.declare_dram_parameter(f"w{i}", [H, H], bf16, isOutput=False)
               for i in (1, 2, 3)]
    b_dram = nc.declare_dram_parameter("b", [4, H], bf16, isOutput=False)
    ko_dram = nc.declare_dram_parameter("ko", [2, 8], u32, isOutput=False)
    out_dram = nc.declare_dram_parameter("out", [L, H], f32, isOutput=True)

    rg = [list(range(NCORES))]

    def allgather(ins_ap, outs_ap):
        nc.gpsimd.collective_compute(
            "AllGather", mybir.AluOpType.bypass, replica_groups=rg,
            ins=[ins_ap], outs=[outs_ap])

    with tile.TileContext(nc) as tc:
        with (
            tc.tile_pool(name="sb", bufs=1) as sb,
            tc.tile_pool(name="zp", bufs=8) as zp,
            tc.tile_pool(name="lz", bufs=2) as lzp,
            tc.tile_pool(name="hp", bufs=4) as hp,
            tc.tile_pool(name="ep", bufs=4) as ep,
            tc.tile_pool(name="ps", bufs=8, space="PSUM") as ps,
            tc.tile_pool(name="dr", bufs=1, space="DRAM") as dr,
        ):
            # ---- persistent SBUF tiles / loads ----
            # z0 + adjacency interleaved so layer 0 can start on chunk 0
            # while the rest still streams in
            a_sb = sb.tile([128, NCH, L], fp8)        # 64 KB/partition
            z0_sb = sb.tile([128, NCH, IN], bf16)     # 16 KB/partition
            for g in range(NCORES):
                nc.sync.dma_start(
                    z0_sb[:, g * RJ:(g + 1) * RJ, :],
                    z0_dram[g * L:(g + 1) * L, :].rearrange(
                        "(c p) w -> p c w", p=128))
                for c in range(RJ):
                    k = g * RJ + c
                    nc.sync.dma_start(a_sb[:, k, :],
                                      a_dram[k * 128:(k + 1) * 128, :])

            rbc = sb.tile([128, L], f32)              # dst scale (has 1/ZS)
            rp = sb.tile([128, RJ], f32)              # src scale (has ZS)
            nc.sync.dma_start(rbc[:], rbc_dram[:])
            nc.sync.dma_start(rp[:], rp_dram[:])

            # per-core gathered-block row offsets (rotated rank order),
            # one row per row group
            koff = []
            for g in range(2):
                cnt = GRP[g][1] - GRP[g][0]
                row = []
                for j in range(NOTH):
                    rko = nc.sync.alloc_register(f"rko{g}_{j}")
                    nc.sync.reg_load(rko, ko_dram[g:g + 1, j:j + 1])
                    row.append(nc.sync.snap(
                        rko, donate=True, min_val=0,
                        max_val=(NCORES - 1) * cnt * 128))
                koff.append(row)

            # ---- weights (needed only once layer 0's aggregation is done) ----
            w0_sb = sb.tile([128, 1, H], bf16)
            wx_sb = [sb.tile([128, 4, H], bf16, name=f"wx{i}") for i in range(3)]
            nc.sync.dma_start(w0_sb[:, 0, :], w0_dram[:])
            for i in range(3):
                for ci in range(4):
                    nc.sync.dma_start(wx_sb[i][:, ci, :],
                                      wx_dram[i][ci * 128:(ci + 1) * 128, :])
            if use_bias:
                b_sb = sb.tile([1, 4, H], bf16)
                ones_row_b = sb.tile([1, 128], bf16)
                nc.vector.memset(ones_row_b[:], 1.0)
                for l in range(4):
                    nc.sync.dma_start(b_sb[:, l, :], b_dram[l:l + 1, :])

            # ---- layer 0 aggregation: all 64 chunks local (z0 replicated,
            # bf16 — slower than fp8 but hidden under the entry barrier) ----
            h0 = [ps.tile([128, 512], f32, tag="acc", name=f"h0_{j}")
                  for j in range(NJ)]
            for c in range(NCH):
                for j in range(NJ):
                    nc.tensor.matmul(
                        h0[j][:],
                        z0_sb[:, c, :],
                        a_sb[:, c, j * 512:(j + 1) * 512],
                        start=(c == 0), stop=(c == NCH - 1))
            hT = [hp.tile([128, 1, L], bf16, tag="hT", name="hT0")]
            for j in range(NJ):
                nc.vector.tensor_mul(hT[0][:, 0, j * 512:(j + 1) * 512],
                                     h0[j][:], rbc[:, j * 512:(j + 1) * 512])

            # ---- per-layer epilogue + next-layer aggregation ----
            def epilogue(layer, hT, lz_next, ag_i, ag_o):
                """full-width weight matmul + softplus in two row groups;
                AllGather each group (all channels) as soon as its 4 row
                chunks are staged — the first collective's input chain is
                only 4 tiles and the next layer's first gathered phase needs
                only that one collective."""
                ci_n = 1 if layer == 0 else 4
                w_l = w0_sb if layer == 0 else wx_sb[layer - 1]
                for g in range(2):
                    for rj in range(GRP[g][0], GRP[g][1]):
                        y_ps = ps.tile([128, H], f32, tag="acc",
                                       name=f"yps{layer}_{rj}")
                        if use_bias:
                            nc.tensor.matmul(y_ps[:], ones_row_b[:],
                                             b_sb[:, layer, :],
                                             start=True, stop=False)
                        for ci in range(ci_n):
                            nc.tensor.matmul(
                                y_ps[:],
                                hT[ci][:, 0, rj * 128:(rj + 1) * 128],
                                w_l[:, ci, :],
                                start=(ci == 0 and not use_bias),
                                stop=(ci == ci_n - 1))
                        # softplus = ln(exp(y) + 1)
                        ey = ep.tile([128, H], f32, tag="ey")
                        nc.scalar.activation(ey[:], y_ps[:], EXP)
                        sp = ep.tile([128, H], f32, tag="sp")
                        nc.scalar.activation(sp[:], ey[:], LN, bias=1.0)
                        if layer < 3:
                            nc.vector.tensor_scalar_mul(
                                lz_next[:, rj, :], sp[:], rp[:, rj:rj + 1])
                            nc.sync.dma_start(
                                ag_i[g][(rj - GRP[g][0]) * 128:
                                        (rj - GRP[g][0] + 1) * 128, :],
                                lz_next[:, rj, :])
                        else:
                            nc.sync.dma_start(
                                out_dram[rj * 128:(rj + 1) * 128, :], sp[:])
                    if layer < 3:
                        allgather(ag_i[g][:], ag_o[g][:])

            lz = None
            zsrc = None
            for layer in range(4):
                if layer < 3:
                    ag_i = [dr.tile([(GRP[g][1] - GRP[g][0]) * 128, H],
                                    fp8, tag=f"agi{layer}_{g}",
                                    name=f"agi{layer}_{g}")
                            for g in range(2)]
                    ag_o = [dr.tile([(GRP[g][1] - GRP[g][0]) * 128 * NCORES,
                                     H], fp8, tag=f"ago{layer}_{g}",
                                    addr_space="Shared",
                                    name=f"ago{layer}_{g}")
                            for g in range(2)]
                    lzn = lzp.tile([128, RJ, H], fp8, tag="lz",
                                   name=f"lz{layer}")
                else:
                    ag_i = ag_o = lzn = None
                epilogue(layer, hT, lzn, ag_i, ag_o)
                if layer == 3:
                    break

                # ---- layer (layer+1) aggregation: the local (own-block)
                # part for all channels is emitted first — it needs no
                # comm and fills the AllGather wait; then two channel-half
                # gathered phases ----
                lz = lzn
                zsrc = ag_o
                hT = [hp.tile([128, 1, L], bf16, tag="hT",
                              name=f"hT{layer + 1}_{x}") for x in range(4)]
                h_ps = [[ps.tile([128, 512], f32, tag="acc",
                                 name=f"hps{layer + 1}_{ci}_{j}")
                         for j in range(NJ)] for ci in range(4)]
                for t in range(RJ // 2):
                    for ci in range(4):
                        for j in range(NJ):
                            nc.tensor.matmul(
                                h_ps[ci][j][:],
                                lz[:, 2 * t:2 * t + 2,
                                   ci * 128:(ci + 1) * 128],
                                a_sb[:, NOTH * RJ + 2 * t:
                                     NOTH * RJ + 2 * t + 2,
                                     j * 512:(j + 1) * 512],
                                start=(t == 0), stop=False,
                                perf_mode=DR)
                for ph in range(2):
                    for g in range(2):
                        g0, g1 = GRP[g]
                        cnt = g1 - g0
                        for j in range(NOTH):
                            zkb = zp.tile([128, cnt, HH], fp8, tag="zkb",
                                          name=f"zkb{g}")
                            nc.sync.dma_start(
                                zkb[:],
                                zsrc[g][ds(koff[g][j], cnt * 128),
                                        ph * HH:(ph + 1) * HH].rearrange(
                                    "(c p) w -> p c w", p=128))
                            for t in range(cnt // 2):
                                last = (g == 1) and (j == NOTH - 1) \
                                    and (t == cnt // 2 - 1)
                                for cl in range(2):
                                    ci = ph * 2 + cl
                                    for nj in range(NJ):
                                        nc.tensor.matmul(
                                            h_ps[ci][nj][:],
                                            zkb[:, 2 * t:2 * t + 2,
                                                cl * 128:(cl + 1) * 128],
                                            a_sb[:, j * RJ + g0 + 2 * t:
                                                 j * RJ + g0 + 2 * t + 2,
                                                 nj * 512:(nj + 1) * 512],
                                            start=False, stop=last,
                                            perf_mode=DR)
                    # evict this phase's channels (dst scale folded in)
                    for cl in range(2):
                        ci = ph * 2 + cl
                        for nj in range(NJ):
                            nc.vector.tensor_mul(
                                hT[ci][:, 0, nj * 512:(nj + 1) * 512],
                                h_ps[ci][nj][:],
                                rbc[:, nj * 512:(nj + 1) * 512])

    nc.compile()
    return nc


def _prep_shards(atom_pos, dist_adj, atom_emb, W0, b0, W1, b1, W2, b2, W3, b3):
    adj = np.asarray(dist_adj, dtype=np.float32).copy()
    np.fill_diagonal(adj, 0.0)          # reference removes self loops
    deg = adj.sum(axis=0)               # symmetric: in-deg == out-deg
    r = 1.0 / np.sqrt(np.maximum(deg, 1.0))
    feat0 = np.concatenate(
        [np.asarray(atom_emb, np.float32), np.asarray(atom_pos, np.float32)],
        axis=1)
    z0 = (ZS * r[:, None] * feat0).astype(BF16)   # pre-scaled input features
    a8 = adj.astype(F8)                           # entries exactly 0/1
    w0 = np.asarray(W0, np.float32).astype(BF16)
    wx = [np.asarray(w, np.float32).astype(BF16) for w in (W1, W2, W3)]
    b = np.stack([np.asarray(x, np.float32) for x in (b0, b1, b2, b3)]
                 ).astype(BF16)
    in_maps = []
    for m in range(NCORES):
        sl = slice(m * L, (m + 1) * L)
        rot = [(m + 1 + j) % NCORES for j in range(NOTH)] + [m]
        rows = np.concatenate([np.arange(rk * L, (rk + 1) * L) for rk in rot])
        r_loc = r[sl].astype(np.float32)
        rbc = np.broadcast_to(r_loc / ZS, (128, L)).copy()   # dst scale
        rp = (ZS * r_loc).reshape(RJ, 128).T.copy()          # src scale
        ko = np.array([[rk * (GRP[g][1] - GRP[g][0]) * 128 for rk in rot]
                       for g in range(2)], dtype=np.uint32)
        im = {"a": np.ascontiguousarray(a8[rows][:, sl]),
              "z0": np.ascontiguousarray(z0[rows]),
              "rbc": rbc, "rp": rp,
              "w0": w0, "w1": wx[0], "w2": wx[1], "w3": wx[2], "b": b,
              "ko": ko}
        in_maps.append(im)
    return in_maps


def kernel(**inputs):
    from concourse.bass_utils import run_bass_kernel_spmd

    use_bias = any(
        np.any(np.asarray(inputs[f"b{i}"]) != 0) for i in range(4))
    key = ("nc", use_bias)
    if key not in _STATE:
        _STATE[key] = _build(use_bias)
    nc = _STATE[key]
    in_maps = _prep_shards(**inputs)
    res = run_bass_kernel_spmd(nc, in_maps, core_ids=list(range(NCORES)))
    out = np.concatenate([res.results[m]["out"] for m in range(NCORES)], axis=0)
    return out.astype(np.float32)


# revision 26
# speedup vs baseline: 1.0293x; 1.0293x over previous
"""AtomPosGNN Trainium2 kernel: 4-layer GraphConv (norm='both') over a dense
0/1 adjacency, SPMD across 8 NeuronCores.

Sharding: nodes split 1024/core. Core m holds the full-height column block
A[:, m*1024:(m+1)*1024] (== row block transposed; A symmetric) as exact 0/1
fp8e4m3, resident in SBUF, rows reordered into rotated rank order
(m+1)%8, ..., (m+7)%8, m (own block last). Layers 1-3 aggregate with fp8
DoubleRow matmuls (two 128-row source chunks per instruction, ~1.5-2x PE
throughput): stationary operand = feature chunk pair [128, 2, 128ch],
moving operand = adjacency chunk pair [128, 2, 512dst].

Host precomputes the degree norms r = rsqrt(max(deg,1)) (graph setup, same
as dgl) and the pre-scaled input features z0 = 16*r*[emb|pos], replicated
to every core in its rotated row order — layer 0 needs no collective and
starts immediately; it runs in bf16 (plain matmuls) since the startup
window is gated by the first collective's entry barrier anyway, and bf16
improves precision for free. Later layers' features are fp8, scaled by 16
to stay out of the subnormal range; the 1/16 is folded into the dst-scale
vector rbc. A tiny dummy AllGather fires first so the one-time CC entry
barrier overlaps the adjacency load and layer 0.

Layers 1-3 gather features via fp8 AllGathers split into two column halves;
each layer's own-block (local) aggregation for all channels is emitted ahead
of the gathered phases so it fills the AllGather wait, and the second half's
collective flies under the first half's aggregation phase. Gathered rank
blocks are fetched with per-core dynamic (register) DMA offsets so every
core skips its own block without branching.

softplus = ln(exp(y)+1) on the ACT engine; the activation-table selection is
patched so both EXP and LN resolve to the combined natural_log_exp table —
one table load for the whole kernel instead of a ~1.3us reload per function
switch.
"""

import numpy as np
import ml_dtypes

N = 8192
NCORES = 8
L = N // NCORES          # 1024 local nodes per core
EMB = 125
POS = 3
IN = 128                 # EMB + POS
H = 512
HH = H // 2              # column half for the split AllGather
RJ = L // 128            # 8 row chunks per rank block
NJ = L // 512            # 2 free-dim chunks of 512 in aggregation
NOTH = NCORES - 1        # 7 gathered (off-rank) blocks
NCH = N // 128           # 64 source chunks total
ZS = 16.0                # fp8 feature scale (keeps z out of subnormals)

BF16 = ml_dtypes.bfloat16
F8 = ml_dtypes.float8_e4m3fn

_STATE = {}


def _patch_act_tables():
    """Make the act-table pass pick the combined exp+ln table for both EXP
    and LN (greedy first-match otherwise alternates two tables, reloading
    ~1.3us per switch). Empties the tables before the combined one so ids
    stay positional."""
    import concourse.bacc as bm
    import concourse.hw_specs as hw
    if getattr(bm, "_gnn_act_patch", False):
        return
    orig = hw.get_activation_tables

    def patched(arch):
        t = orig(arch)
        names = list(t.keys())
        if "natural_log_exp_and_others" not in names:
            return t
        i = names.index("natural_log_exp_and_others")
        return {n: (set() if k < i else t[n]) for k, n in enumerate(names)}

    bm.get_activation_tables = patched
    bm._gnn_act_patch = True


def _build(use_bias):
    import concourse.bass as bass
    import concourse.mybir as mybir
    import concourse.tile as tile
    from concourse import bacc
    from concourse.bass import ds

    _patch_act_tables()

    f32 = mybir.dt.float32
    bf16 = mybir.dt.bfloat16
    fp8 = mybir.dt.float8e4
    u32 = mybir.dt.uint32
    EXP = mybir.ActivationFunctionType.Exp
    LN = mybir.ActivationFunctionType.Ln
    DR = mybir.MatmulPerfMode.DoubleRow

    nc = bacc.Bacc("TRN2", target_bir_lowering=False, debug=False,
                   num_devices=NCORES)

    a_dram = nc.declare_dram_parameter("a", [N, L], fp8, isOutput=False)
    z0_dram = nc.declare_dram_parameter("z0", [N, IN], bf16, isOutput=False)
    rbc_dram = nc.declare_dram_parameter("rbc", [128, L], f32, isOutput=False)
    rp_dram = nc.declare_dram_parameter("rp", [128, RJ], f32, isOutput=False)
    w0_dram = nc.declare_dram_parameter("w0", [IN, H], bf16, isOutput=False)
    wx_dram = [nc.declare_dram_parameter(f"w{i}", [H, H], bf16, isOutput=False)
               for i in (1, 2, 3)]
    b_dram = nc.declare_dram_parameter("b", [4, H], bf16, isOutput=False)
    ko_dram = nc.declare_dram_parameter("ko", [1, 8], u32, isOutput=False)
    out_dram = nc.declare_dram_parameter("out", [L, H], f32, isOutput=True)

    rg = [list(range(NCORES))]

    def allgather(ins_ap, outs_ap):
        nc.gpsimd.collective_compute(
            "AllGather", mybir.AluOpType.bypass, replica_groups=rg,
            ins=[ins_ap], outs=[outs_ap])

    with tile.TileContext(nc) as tc:
        with (
            tc.tile_pool(name="sb", bufs=1) as sb,
            tc.tile_pool(name="zp", bufs=8) as zp,
            tc.tile_pool(name="lz", bufs=2) as lzp,
            tc.tile_pool(name="hp", bufs=4) as hp,
            tc.tile_pool(name="ep", bufs=4) as ep,
            tc.tile_pool(name="ps", bufs=8, space="PSUM") as ps,
            tc.tile_pool(name="dr", bufs=1, space="DRAM") as dr,
        ):
            # ---- persistent SBUF tiles / loads ----
            # z0 + adjacency interleaved so layer 0 can start on chunk 0
            # while the rest still streams in
            a_sb = sb.tile([128, NCH, L], fp8)        # 64 KB/partition
            z0_sb = sb.tile([128, NCH, IN], bf16)     # 16 KB/partition
            for g in range(NCORES):
                nc.sync.dma_start(
                    z0_sb[:, g * RJ:(g + 1) * RJ, :],
                    z0_dram[g * L:(g + 1) * L, :].rearrange(
                        "(c p) w -> p c w", p=128))
                for c in range(RJ):
                    k = g * RJ + c
                    nc.sync.dma_start(a_sb[:, k, :],
                                      a_dram[k * 128:(k + 1) * 128, :])

            rbc = sb.tile([128, L], f32)              # dst scale (has 1/ZS)
            rp = sb.tile([128, RJ], f32)              # src scale (has ZS)
            nc.sync.dma_start(rbc[:], rbc_dram[:])
            nc.sync.dma_start(rp[:], rp_dram[:])

            # per-core gathered-block row offsets (rotated rank order)
            koff = []
            for j in range(NOTH):
                rko = nc.sync.alloc_register(f"rko{j}")
                nc.sync.reg_load(rko, ko_dram[0:1, j:j + 1])
                koff.append(nc.sync.snap(rko, donate=True, min_val=0,
                                         max_val=N // 2 - L // 2))

            # ---- weights (needed only once layer 0's aggregation is done) ----
            w0_sb = sb.tile([128, 1, H], bf16)
            wx_sb = [sb.tile([128, 4, H], bf16, name=f"wx{i}") for i in range(3)]
            nc.sync.dma_start(w0_sb[:, 0, :], w0_dram[:])
            for i in range(3):
                for ci in range(4):
                    nc.sync.dma_start(wx_sb[i][:, ci, :],
                                      wx_dram[i][ci * 128:(ci + 1) * 128, :])
            if use_bias:
                b_sb = sb.tile([1, 4, H], bf16)
                ones_row_b = sb.tile([1, 128], bf16)
                nc.vector.memset(ones_row_b[:], 1.0)
                for l in range(4):
                    nc.sync.dma_start(b_sb[:, l, :], b_dram[l:l + 1, :])

            # ---- layer 0 aggregation: all 64 chunks local (z0 replicated,
            # bf16 — slower than fp8 but hidden under the entry barrier) ----
            h0 = [ps.tile([128, 512], f32, tag="acc", name=f"h0_{j}")
                  for j in range(NJ)]
            for c in range(NCH):
                for j in range(NJ):
                    nc.tensor.matmul(
                        h0[j][:],
                        z0_sb[:, c, :],
                        a_sb[:, c, j * 512:(j + 1) * 512],
                        start=(c == 0), stop=(c == NCH - 1))
            hT = [hp.tile([128, 1, L], bf16, tag="hT", name="hT0")]
            for j in range(NJ):
                nc.vector.tensor_mul(hT[0][:, 0, j * 512:(j + 1) * 512],
                                     h0[j][:], rbc[:, j * 512:(j + 1) * 512])

            # ---- per-layer epilogue + next-layer aggregation ----
            def epilogue(layer, hT, lz_next, ag_i, ag_o):
                """full-width weight matmul + softplus in two row groups;
                AllGather each group (all channels) as soon as its 4 row
                chunks are staged — the first collective's input chain is
                only 4 tiles and the next layer's first gathered phase needs
                only that one collective."""
                ci_n = 1 if layer == 0 else 4
                w_l = w0_sb if layer == 0 else wx_sb[layer - 1]
                for g in range(2):
                    for rj in range(g * 4, g * 4 + 4):
                        y_ps = ps.tile([128, H], f32, tag="acc",
                                       name=f"yps{layer}_{rj}")
                        if use_bias:
                            nc.tensor.matmul(y_ps[:], ones_row_b[:],
                                             b_sb[:, layer, :],
                                             start=True, stop=False)
                        for ci in range(ci_n):
                            nc.tensor.matmul(
                                y_ps[:],
                                hT[ci][:, 0, rj * 128:(rj + 1) * 128],
                                w_l[:, ci, :],
                                start=(ci == 0 and not use_bias),
                                stop=(ci == ci_n - 1))
                        # softplus = ln(exp(y) + 1)
                        ey = ep.tile([128, H], f32, tag="ey")
                        nc.scalar.activation(ey[:], y_ps[:], EXP)
                        sp = ep.tile([128, H], f32, tag="sp")
                        nc.scalar.activation(sp[:], ey[:], LN, bias=1.0)
                        if layer < 3:
                            nc.vector.tensor_scalar_mul(
                                lz_next[:, rj, :], sp[:], rp[:, rj:rj + 1])
                            nc.sync.dma_start(
                                ag_i[g][(rj - g * 4) * 128:
                                        (rj - g * 4 + 1) * 128, :],
                                lz_next[:, rj, :])
                        else:
                            nc.sync.dma_start(
                                out_dram[rj * 128:(rj + 1) * 128, :], sp[:])
                    if layer < 3:
                        allgather(ag_i[g][:], ag_o[g][:])

            lz = None
            zsrc = None
            for layer in range(4):
                if layer < 3:
                    ag_i = [dr.tile([L // 2, H], fp8,
                                    tag=f"agi{layer}_{g}",
                                    name=f"agi{layer}_{g}")
                            for g in range(2)]
                    ag_o = [dr.tile([N // 2, H], fp8,
                                    tag=f"ago{layer}_{g}",
                                    addr_space="Shared",
                                    name=f"ago{layer}_{g}")
                            for g in range(2)]
                    lzn = lzp.tile([128, RJ, H], fp8, tag="lz",
                                   name=f"lz{layer}")
                else:
                    ag_i = ag_o = lzn = None
                epilogue(layer, hT, lzn, ag_i, ag_o)
                if layer == 3:
                    break

                # ---- layer (layer+1) aggregation: the local (own-block)
                # part for all channels is emitted first — it needs no
                # comm and fills the AllGather wait; then two channel-half
                # gathered phases ----
                lz = lzn
                zsrc = ag_o
                hT = [hp.tile([128, 1, L], bf16, tag="hT",
                              name=f"hT{layer + 1}_{x}") for x in range(4)]
                h_ps = [[ps.tile([128, 512], f32, tag="acc",
                                 name=f"hps{layer + 1}_{ci}_{j}")
                         for j in range(NJ)] for ci in range(4)]
                for t in range(RJ // 2):
                    for ci in range(4):
                        for j in range(NJ):
                            nc.tensor.matmul(
                                h_ps[ci][j][:],
                                lz[:, 2 * t:2 * t + 2,
                                   ci * 128:(ci + 1) * 128],
                                a_sb[:, NOTH * RJ + 2 * t:
                                     NOTH * RJ + 2 * t + 2,
                                     j * 512:(j + 1) * 512],
                                start=(t == 0), stop=False,
                                perf_mode=DR)
                for ph in range(2):
                    for g in range(2):
                        for j in range(NOTH):
                            zkb = zp.tile([128, RJ // 2, HH], fp8, tag="zkb")
                            nc.sync.dma_start(
                                zkb[:],
                                zsrc[g][ds(koff[j], L // 2),
                                        ph * HH:(ph + 1) * HH].rearrange(
                                    "(c p) w -> p c w", p=128))
                            for t in range(2):
                                last = (g == 1) and (j == NOTH - 1) and (t == 1)
                                for cl in range(2):
                                    ci = ph * 2 + cl
                                    for nj in range(NJ):
                                        nc.tensor.matmul(
                                            h_ps[ci][nj][:],
                                            zkb[:, 2 * t:2 * t + 2,
                                                cl * 128:(cl + 1) * 128],
                                            a_sb[:, j * RJ + g * 4 + 2 * t:
                                                 j * RJ + g * 4 + 2 * t + 2,
                                                 nj * 512:(nj + 1) * 512],
                                            start=False, stop=last,
                                            perf_mode=DR)
                    # evict this phase's channels (dst scale folded in)
                    for cl in range(2):
                        ci = ph * 2 + cl
                        for nj in range(NJ):
                            nc.vector.tensor_mul(
                                hT[ci][:, 0, nj * 512:(nj + 1) * 512],
                                h_ps[ci][nj][:],
                                rbc[:, nj * 512:(nj + 1) * 512])

    nc.compile()
    return nc


def _prep_shards(atom_pos, dist_adj, atom_emb, W0, b0, W1, b1, W2, b2, W3, b3):
    adj = np.asarray(dist_adj, dtype=np.float32).copy()
    np.fill_diagonal(adj, 0.0)          # reference removes self loops
    deg = adj.sum(axis=0)               # symmetric: in-deg == out-deg
    r = 1.0 / np.sqrt(np.maximum(deg, 1.0))
    feat0 = np.concatenate(
        [np.asarray(atom_emb, np.float32), np.asarray(atom_pos, np.float32)],
        axis=1)
    z0 = (ZS * r[:, None] * feat0).astype(BF16)   # pre-scaled input features
    a8 = adj.astype(F8)                           # entries exactly 0/1
    w0 = np.asarray(W0, np.float32).astype(BF16)
    wx = [np.asarray(w, np.float32).astype(BF16) for w in (W1, W2, W3)]
    b = np.stack([np.asarray(x, np.float32) for x in (b0, b1, b2, b3)]
                 ).astype(BF16)
    in_maps = []
    for m in range(NCORES):
        sl = slice(m * L, (m + 1) * L)
        rot = [(m + 1 + j) % NCORES for j in range(NOTH)] + [m]
        rows = np.concatenate([np.arange(rk * L, (rk + 1) * L) for rk in rot])
        r_loc = r[sl].astype(np.float32)
        rbc = np.broadcast_to(r_loc / ZS, (128, L)).copy()   # dst scale
        rp = (ZS * r_loc).reshape(RJ, 128).T.copy()          # src scale
        ko = np.array([[rk * (L // 2) for rk in rot]], dtype=np.uint32)
        im = {"a": np.ascontiguousarray(a8[rows][:, sl]),
              "z0": np.ascontiguousarray(z0[rows]),
              "rbc": rbc, "rp": rp,
              "w0": w0, "w1": wx[0], "w2": wx[1], "w3": wx[2], "b": b,
              "ko": ko}
        in_maps.append(im)
    return in_maps


def kernel(**inputs):
    from concourse.bass_utils import run_bass_kernel_spmd

    use_bias = any(
        np.any(np.asarray(inputs[f"b{i}"]) != 0) for i in range(4))
    key = ("nc", use_bias)
    if key not in _STATE:
        _STATE[key] = _build(use_bias)
    nc = _STATE[key]
    in_maps = _prep_shards(**inputs)
    res = run_bass_kernel_spmd(nc, in_maps, core_ids=list(range(NCORES)))
    out = np.concatenate([res.results[m]["out"] for m in range(NCORES)], axis=0)
    return out.astype(np.float32)
